# revision 9
# baseline (speedup 1.0000x reference)
"""Multi-layer GCN (2x GCNConv + linear head) on 8 Trainium2 NeuronCores.

Strategy (graph/data parallel, node-sharded):
  - Nodes are partitioned contiguously across the 8 cores (6250 each).
  - Each core aggregates messages for its own dst nodes. Edges are bucketed
    by dst tile (128 dsts) on the host and sorted within the tile.
  - The scatter-add (segment sum) runs on the TensorEngine: for each
    128-edge block, a host-built one-hot scatter matrix S[e, d] =
    (d == dst_e) * w_e (w_e = deg_isqrt[src] * deg_isqrt[dst], the full GCN
    normalization) is contracted against the message block; PSUM
    accumulates across blocks into a feature-major agg^T tile. All S
    matrices are precomputed on the host and streamed from HBM, so the
    TensorEngine consumes whole tiles with one semaphore per tile and no
    per-block compute-engine dependency (keeps the PE HAM un-throttled).
  - Layer 1 needs no on-device gather: source messages are pre-gathered on
    the host and interleaved with their S blocks in one [128, NB1, 256] f16
    tensor (G row ++ S row), streamed per tile with a single HWDGE DMA.
  - Layer 2 gathers h1 rows with dma_gather (batched indirect DMA) from
    AllGathered tables in HBM, with gather calls cycled over all 4 SWDGE
    queues so Q7 descriptor generation uses all 8 GpSimd cores.
  - Self loops: each tile's own rows are a contiguous HWDGE DMA load,
    accumulated via a matmul against a prebuilt diagonal bank
    diag(deg^-1[dst]) (49 tiles built once on the ScalarEngine).
  - The inter-layer AllGather is split in 2 by source-node range, each
    chunk fired as soon as layer 1 finishes its tiles, so collectives and
    layer-2 gather descriptor generation overlap layer 1's tail. Layer-2
    edges are grouped by source range into 2 groups per tile; each group's
    dma_gather indexes its own table (int16-safe).
  - Layer weights are applied right on the feature-major agg tiles; layer-1
    output is transposed back to node-major (TensorE transpose) and written
    to HBM for the chunked AllGather. Layer-2 output stays feature-major
    and feeds the output projection directly (lhsT = h2^T), producing
    node-major [dst, 64] tiles.
"""

import sys

sys.path.insert(0, "/opt/trn_rl_repo")

import numpy as np

N = 50000
C_IN = 128
HID = 128
C_OUT = 64
NCORES = 8
NPER = N // NCORES
P = 128
NT = (NPER + P - 1) // P

NGRP = 2
TB = [0, 24, NT]                          # tile boundaries of the groups
RS = [TB[g] * P for g in range(NGRP)]     # row range [RS, RE) of group g
RE = [min(TB[g + 1] * P, NPER) for g in range(NGRP)]
RG = [RE[g] - RS[g] for g in range(NGRP)]
assert max(RG) * NCORES < 32768  # gather idxs are int16

MAXIDX = 1024    # max idxs per dma_gather call (larger calls fault)
NQ = 4           # SWDGE queues to cycle gather calls over

LAST_RESULT = None  # BassKernelResults of the most recent run (for test.py)


def _r16(n):
    return (int(n) + 15) // 16 * 16


def _preprocess(edge_index, x, W1, b1, W2, b2, Wo, bo):
    """Host-side graph preprocessing -> per-core input arrays + schedule."""
    src_e = np.asarray(edge_index[0], np.int64)
    dst_e = np.asarray(edge_index[1], np.int64)
    # degree includes the self loop
    deg = (np.bincount(dst_e, minlength=N) + 1).astype(np.float32)
    disqrt = (1.0 / np.sqrt(deg)).astype(np.float32)

    xh = np.asarray(x, np.float32).astype(np.float16)

    # per (core, tile) edge buckets (no self loops), and per-group subsets
    per_core = []    # [c][t] -> (srcs, dds)  (layer-1 flat order)
    per_core_g = []  # [c][t][g] -> (srcs, pos_in_table, dds)
    n1 = np.zeros((NCORES, NT), np.int64)
    ng = np.zeros((NCORES, NT, NGRP), np.int64)
    for c in range(NCORES):
        m = (dst_e >= c * NPER) & (dst_e < (c + 1) * NPER)
        s_c = src_e[m]
        d_c = dst_e[m] - c * NPER
        order = np.argsort(d_c, kind="stable")
        s_c, d_c = s_c[order], d_c[order]
        bounds = np.searchsorted(d_c, np.arange(0, NT + 1) * P)
        tiles, tiles_g = [], []
        for t in range(NT):
            ss = s_c[bounds[t]:bounds[t + 1]]
            dd = d_c[bounds[t]:bounds[t + 1]] - t * P
            tiles.append((ss, dd))
            n1[c, t] = len(ss)
            cc, rr = ss // NPER, ss % NPER
            gl = []
            for g in range(NGRP):
                sel = (rr >= RS[g]) & (rr < RE[g])
                pos = cc[sel] * RG[g] + (rr[sel] - RS[g])
                gl.append((ss[sel], pos, dd[sel]))
                ng[c, t, g] = sel.sum()
            tiles_g.append(gl)
        per_core.append(tiles)
        per_core_g.append(tiles_g)

    # layer-1 schedule: one group per tile, block-padded
    M1 = [int(n1[:, t].max()) for t in range(NT)]
    B1 = [(m + P - 1) // P for m in M1]
    NB1 = int(sum(B1))
    # layer-2 schedule: NGRP groups per tile, 16-granular idx padding
    MG = [[_r16(ng[:, t, g].max()) for g in range(NGRP)] for t in range(NT)]
    BG = [[(MG[t][g] + P - 1) // P for g in range(NGRP)] for t in range(NT)]
    NB2 = int(sum(sum(bg) for bg in BG))
    NC16 = int(sum(sum(mg) for mg in MG)) // 16
    tile_ws = [min(P, NPER - t * P) for t in range(NT)]

    in_maps = []
    for c in range(NCORES):
        # ---- layer 1: pregathered messages interleaved with S blocks -----
        gs1 = np.zeros((P, NB1, C_IN + P), np.float16)
        blk = 0
        for t in range(NT):
            ss, dd = per_core[c][t]
            nb = B1[t]
            if nb == 0:
                continue
            n = len(ss)
            w = disqrt[dd + t * P + c * NPER] * disqrt[ss]
            ei = np.arange(n)
            bi = ei // P + blk
            pi = ei % P
            gs1[pi, bi, C_IN + dd] = w.astype(np.float16)
            flat_g = np.zeros((nb * P, C_IN), np.float16)
            flat_g[:n] = xh[ss]
            gs1[:, blk:blk + nb, :C_IN] = \
                flat_g.reshape(nb, P, C_IN).transpose(1, 0, 2)
            blk += nb
        assert blk == NB1

        # ---- layer 2: gather idxs + streamed S blocks --------------------
        idx16 = np.zeros((16, NC16), np.int16)
        sblk = np.zeros((P, NB2, P), np.float16)
        col16 = 0
        blk = 0
        for t in range(NT):
            for g in range(NGRP):
                ss, pos, dd = per_core_g[c][t][g]
                m_pad = MG[t][g]
                nb = BG[t][g]
                if m_pad == 0:
                    continue
                n = len(ss)
                flat_i = np.zeros(m_pad, np.int16)
                flat_i[:n] = pos.astype(np.int16)
                idx16[:, col16:col16 + m_pad // 16] = \
                    flat_i.reshape(m_pad // 16, 16).T
                col16 += m_pad // 16
                w = disqrt[dd + t * P + c * NPER] * disqrt[ss]
                ei = np.arange(n)
                bi = ei // P + blk
                sblk[ei % P, bi, dd] = w.astype(np.float16)
                blk += nb
        assert col16 == NC16 and blk == NB2
        idx_full = np.tile(idx16, (8, 1)).astype(np.int16)

        # self-loop scale: deg^-1 of each dst (deg_isqrt^2)
        dsqnm = np.zeros((P, NT), np.float32)
        for t in range(NT):
            tw = tile_ws[t]
            dv = disqrt[c * NPER + t * P: c * NPER + t * P + tw]
            dsqnm[:tw, t] = dv * dv

        in_maps.append({
            "gs1": gs1,
            "sblk": sblk,
            "xss": xh[c * NPER:(c + 1) * NPER].copy(),
            "idx": idx_full,
            "dsqnm": dsqnm,
            "w1": np.asarray(W1, np.float32).astype(np.float16),
            "w2": np.asarray(W2, np.float32).astype(np.float16),
            "wo": np.asarray(Wo, np.float32).astype(np.float16),
            "b1": np.asarray(b1, np.float32).reshape(HID, 1).copy(),
            "b2": np.asarray(b2, np.float32).reshape(HID, 1).copy(),
            "bo": np.tile(np.asarray(bo, np.float32)[None, :], (P, 1)),
        })

    sched = dict(M1=M1, B1=B1, NB1=NB1, MG=MG, BG=BG, NB2=NB2,
                 NC16=NC16, tile_ws=tile_ws)
    return in_maps, sched


def _build_program(sched):
    import concourse.bass as bass
    import concourse.bacc as bacc
    import concourse.tile as tile
    import concourse.mybir as mybir
    from concourse.masks import make_identity

    f32 = mybir.dt.float32
    f16 = mybir.dt.float16
    i16 = mybir.dt.int16
    M1, B1, NB1 = sched["M1"], sched["B1"], sched["NB1"]
    MG, BG, NB2 = sched["MG"], sched["BG"], sched["NB2"]
    NC16, tile_ws = sched["NC16"], sched["tile_ws"]
    nb1_max = max(B1)
    nb2_max = max(sum(bg) for bg in BG)
    nbg_max = [max(BG[t][g] for t in range(NT)) for g in range(NGRP)]
    W = C_IN + P

    nc = bacc.Bacc("TRN2", target_bir_lowering=False, debug=False,
                   num_devices=NCORES, num_swdge_queues=NQ,
                   dynamic_dma_scratch_size=49152)

    gs1_d = nc.dram_tensor("gs1", [P, NB1, W], f16, kind="ExternalInput")
    sblk_d = nc.dram_tensor("sblk", [P, NB2, P], f16, kind="ExternalInput")
    xss_d = nc.dram_tensor("xss", [NPER, C_IN], f16, kind="ExternalInput")
    idx_d = nc.dram_tensor("idx", [P, NC16], i16, kind="ExternalInput")
    dsqnm_d = nc.dram_tensor("dsqnm", [P, NT], f32, kind="ExternalInput")
    w1_d = nc.dram_tensor("w1", [C_IN, HID], f16, kind="ExternalInput")
    w2_d = nc.dram_tensor("w2", [HID, HID], f16, kind="ExternalInput")
    wo_d = nc.dram_tensor("wo", [HID, C_OUT], f16, kind="ExternalInput")
    b1_d = nc.dram_tensor("b1", [HID, 1], f32, kind="ExternalInput")
    b2_d = nc.dram_tensor("b2", [HID, 1], f32, kind="ExternalInput")
    bo_d = nc.dram_tensor("bo", [P, C_OUT], f32, kind="ExternalInput")
    out_d = nc.dram_tensor("out", [NPER, C_OUT], f32, kind="ExternalOutput")

    with tile.TileContext(nc) as tc:
        with tc.tile_pool(name="const", bufs=1) as cpool, \
             tc.tile_pool(name="gs1p", bufs=4) as gs1pool, \
             tc.tile_pool(name="gl0", bufs=8) as gp0, \
             tc.tile_pool(name="gl1", bufs=8) as gp1, \
             tc.tile_pool(name="swide", bufs=4) as swpool, \
             tc.tile_pool(name="work", bufs=6) as wpool, \
             tc.tile_pool(name="psA", bufs=4, space="PSUM") as psA, \
             tc.tile_pool(name="psH", bufs=2, space="PSUM") as psH, \
             tc.tile_pool(name="psT", bufs=2, space="PSUM") as psT, \
             tc.tile_pool(name="dram", bufs=1, space="DRAM") as dram:
            gpools = [gp0, gp1]

            def cload(name, dram_t, shape, dt):
                t = cpool.tile(shape, dt, name=name)
                nc.sync.dma_start(t[:], dram_t[tuple(slice(0, s) for s in shape)])
                return t

            idx_sb = cload("idx_sb", idx_d, [P, NC16], i16)
            dsqnm_sb = cload("dsqnm_sb", dsqnm_d, [P, NT], f32)
            w1_sb = cload("w1_sb", w1_d, [C_IN, HID], f16)
            w2_sb = cload("w2_sb", w2_d, [HID, HID], f16)
            wo_sb = cload("wo_sb", wo_d, [HID, C_OUT], f16)
            b1_sb = cload("b1_sb", b1_d, [HID, 1], f32)
            b2_sb = cload("b2_sb", b2_d, [HID, 1], f32)
            bo_sb = cload("bo_sb", bo_d, [P, C_OUT], f32)

            ident_sb = cpool.tile([P, P], f16, name="ident_sb")
            make_identity(nc, ident_sb[:])

            # prebuilt diagonal bank: diag(deg^-1) per tile, built once on
            # the ScalarEngine and reused by both layers' self-loop matmuls
            dbank = cpool.tile([P, NT, P], f16, name="dbank")
            for t in range(NT):
                nc.scalar.activation(dbank[:, t, :], ident_sb[:, :],
                                     mybir.ActivationFunctionType.Copy,
                                     scale=dsqnm_sb[:, t:t + 1])

            h1s = dram.tile([NPER, HID], f16, name="h1s")
            h1f = [dram.tile([NCORES * RG[g], HID], f16, name=f"h1f{g}",
                             addr_space="Shared") for g in range(NGRP)]

            # register cache for num_idxs_reg values
            regs = {}

            def reg_of(v):
                if v not in regs:
                    regs[v] = nc.gpsimd.to_reg(v)
                return regs[v]

            qctr = [0]

            def next_q():
                q = qctr[0] % NQ
                qctr[0] += 1
                return q

            def tail_a(t, tw, pa, w_sb, b_sb):
                """agg copy + weight matmul + relu (PE stays busy on the
                next tile's aggregation while DVE/ScalarE run these)."""
                agg = wpool.tile([P, tw], f16, tag="agg", name="agg")
                nc.vector.tensor_copy(agg[:], pa[:])
                ph = psH.tile([P, tw], f32, tag="ph", name="ph")
                nc.tensor.matmul(ph[:], lhsT=w_sb[:], rhs=agg[:],
                                 start=True, stop=True)
                h = wpool.tile([P, tw], f16, tag="h", name="h")
                nc.scalar.activation(h[:], ph[:],
                                     mybir.ActivationFunctionType.Relu,
                                     bias=b_sb[:, 0:1])
                return h

            def tail_b(t, tw, h, phase):
                if phase == 0:
                    pt = psT.tile([P, P], f16, tag="pt", name="pt")
                    nc.tensor.transpose(out=pt[:tw, :], in_=h[:, :tw],
                                        identity=ident_sb[:])
                    hn = wpool.tile([P, P], f16, tag="hn", name="hn")
                    nc.vector.tensor_copy(hn[:tw, :], pt[:tw, :])
                    nc.sync.dma_start(h1s[t * P:t * P + tw, :], hn[:tw, :])
                    # fire the AllGather chunk as soon as its rows are done
                    for g in range(NGRP):
                        if t == TB[g + 1] - 1:
                            nc.gpsimd.collective_compute(
                                "AllGather", mybir.AluOpType.bypass,
                                replica_groups=[list(range(NCORES))],
                                ins=[h1s[RS[g]:RE[g], :].opt()],
                                outs=[h1f[g][:].opt()])
                else:
                    po = psT.tile([P, C_OUT], f32, tag="pt", name="po")
                    nc.tensor.matmul(po[:tw, :], lhsT=h[:, :tw],
                                     rhs=wo_sb[:], start=True, stop=True)
                    ob = wpool.tile([P, C_OUT], f32, tag="ob", name="ob")
                    nc.vector.tensor_tensor(out=ob[:tw, :], in0=po[:tw, :],
                                            in1=bo_sb[:tw, :],
                                            op=mybir.AluOpType.add)
                    nc.sync.dma_start(out_d[t * P:t * P + tw, :],
                                      ob[:tw, :])

            def agg0(t, blk):
                tw = tile_ws[t]
                nblk = B1[t]
                GS = gs1pool.tile([P, nb1_max, W], f16, tag="GS", name="GS")
                nc.sync.dma_start(GS[:, 0:nblk, :],
                                  gs1_d[:, blk:blk + nblk, :])
                pa = psA.tile([P, tw], f32, tag="pa", name="pa")
                slab = wpool.tile([P, C_IN], f16, tag="slab", name="slab")
                nc.scalar.dma_start(slab[:tw, :], xss_d[t * P:t * P + tw, :])
                nc.tensor.matmul(pa[:], lhsT=slab[:tw, :],
                                 rhs=dbank[:tw, t, :tw], start=True,
                                 stop=False)
                ks = [P] * (nblk - 1) + [M1[t] - (nblk - 1) * P]
                for j in range(nblk):
                    nc.tensor.matmul(pa[:], lhsT=GS[:ks[j], j, 0:C_IN],
                                     rhs=GS[:ks[j], j, C_IN:C_IN + tw],
                                     start=False, stop=(j == nblk - 1))
                return pa, blk + nblk

            def agg1(t, blk, col16):
                tw = tile_ws[t]
                Gs = []
                for g in range(NGRP):
                    m_pad = MG[t][g]
                    Gg = gpools[g].tile([P, nbg_max[g], C_IN], f16,
                                        tag=f"G{g}", name=f"G{g}")
                    Gs.append(Gg)
                    for o in range(0, m_pad, MAXIDX):
                        n_call = min(MAXIDX, m_pad - o)
                        c0 = col16 + o // 16
                        nc.gpsimd.dma_gather(
                            out_ap=Gg[:, o // P:o // P + (n_call + P - 1) // P, :],
                            in_ap=h1f[g][:, :],
                            idxs_ap=idx_sb[:, c0:c0 + (n_call + 15) // 16],
                            num_idxs=n_call,
                            num_idxs_reg=reg_of(n_call),
                            elem_size=C_IN,
                            queue_num=next_q())
                    col16 += m_pad // 16
                nblk = sum(BG[t])
                swide = swpool.tile([P, nb2_max, P], f16, tag="SW", name="SW")
                nc.scalar.dma_start(swide[:, 0:nblk, :],
                                    sblk_d[:, blk:blk + nblk, :])
                pa = psA.tile([P, tw], f32, tag="pa", name="pa")
                slab = wpool.tile([P, C_IN], f16, tag="slab", name="slab")
                nc.scalar.dma_start(slab[:tw, :], h1s[t * P:t * P + tw, :])
                nc.tensor.matmul(pa[:], lhsT=slab[:tw, :],
                                 rhs=dbank[:tw, t, :tw], start=True,
                                 stop=False)
                j = 0
                for g in range(NGRP):
                    nb = BG[t][g]
                    for jj in range(nb):
                        k = P if jj < nb - 1 else MG[t][g] - (nb - 1) * P
                        nc.tensor.matmul(pa[:], lhsT=Gs[g][:k, jj, :],
                                         rhs=swide[:k, j, :tw],
                                         start=False, stop=(j == nblk - 1))
                        j += 1
                return pa, blk + nblk

            def layer(phase):
                """Two-stage software pipeline: the PE's aggregation chain
                for tile t runs while tile t-1's tail_a (DVE/ScalarE) and
                tile t-2's tail_b latencies hide behind it."""
                agg = agg0 if phase == 0 else agg1
                w_sb = w1_sb if phase == 0 else w2_sb
                b_sb = b1_sb if phase == 0 else b2_sb
                blk = 0
                col16 = 0
                pend_a = None  # (t, tw, pa)
                pend_b = None  # (t, tw, h)
                for t in range(NT):
                    if phase == 0:
                        pa, blk = agg0(t, blk)
                    else:
                        pa, blk = agg1(t, blk, col16)
                        col16 += sum(MG[t]) // 16
                    if pend_a is not None:
                        ta, twa, paa = pend_a
                        h = tail_a(ta, twa, paa, w_sb, b_sb)
                        if pend_b is not None:
                            tb, twb, hb = pend_b
                            tail_b(tb, twb, hb, phase)
                        pend_b = (ta, twa, h)
                    pend_a = (t, tile_ws[t], pa)
                ta, twa, paa = pend_a
                h = tail_a(ta, twa, paa, w_sb, b_sb)
                tb, twb, hb = pend_b
                tail_b(tb, twb, hb, phase)
                tail_b(ta, twa, h, phase)

            layer(0)
            layer(1)

    nc.compile()
    return nc


def kernel(x, edge_index, W1, b1, W2, b2, Wo, bo):
    global LAST_RESULT
    from concourse import bass_utils

    in_maps, sched = _preprocess(edge_index, x, W1, b1, W2, b2, Wo, bo)
    nc = _build_program(sched)
    res = bass_utils.run_bass_kernel_spmd(nc, in_maps,
                                          core_ids=list(range(NCORES)))
    LAST_RESULT = res
    out = np.concatenate([res.results[c]["out"] for c in range(NCORES)], axis=0)
    return out.astype(np.float32)


# revision 10
# speedup vs baseline: 1.0484x; 1.0484x over previous
"""Multi-layer GCN (2x GCNConv + linear head) on 8 Trainium2 NeuronCores.

Strategy (graph/data parallel, node-sharded):
  - Nodes are partitioned contiguously across the 8 cores (6250 each).
  - Each core aggregates messages for its own dst nodes. Edges are bucketed
    by dst tile (128 dsts) on the host and sorted within the tile.
  - The scatter-add (segment sum) runs on the TensorEngine: for each
    128-edge block, a host-built one-hot scatter matrix S[e, d] =
    (d == dst_e) * w_e (w_e = deg_isqrt[src] * deg_isqrt[dst], the full GCN
    normalization) is contracted against the message block; PSUM
    accumulates across blocks into a feature-major agg^T tile. All S
    matrices are precomputed on the host and streamed from HBM, so the
    TensorEngine consumes whole tiles with one semaphore per tile and no
    per-block compute-engine dependency (keeps the PE HAM un-throttled).
  - Layer 1 needs no on-device gather: source messages are pre-gathered on
    the host and interleaved with their S blocks in one [128, NB1, 256] f16
    tensor (G row ++ S row), streamed per tile with a single HWDGE DMA.
  - Layer 2 gathers h1 rows with dma_gather (batched indirect DMA) from
    AllGathered tables in HBM, with gather calls cycled over all 4 SWDGE
    queues so Q7 descriptor generation uses all 8 GpSimd cores.
  - Self loops: each tile's own rows are a contiguous HWDGE DMA load,
    accumulated via a matmul against a prebuilt diagonal bank
    diag(deg^-1[dst]) (49 tiles built once on the ScalarEngine).
  - The inter-layer AllGather is split in 2 by source-node range, each
    chunk fired as soon as layer 1 finishes its tiles, so collectives and
    layer-2 gather descriptor generation overlap layer 1's tail. Layer-2
    edges are grouped by source range into 2 groups per tile; each group's
    dma_gather indexes its own table (int16-safe).
  - Layer weights are applied right on the feature-major agg tiles; layer-1
    output is transposed back to node-major (TensorE transpose) and written
    to HBM for the chunked AllGather. Layer-2 output stays feature-major
    and feeds the output projection directly (lhsT = h2^T), producing
    node-major [dst, 64] tiles.
"""

import sys

sys.path.insert(0, "/opt/trn_rl_repo")

import numpy as np

N = 50000
C_IN = 128
HID = 128
C_OUT = 64
NCORES = 8
NPER = N // NCORES
P = 128
NT = (NPER + P - 1) // P

NGRP = 2
TB = [0, 24, NT]                          # tile boundaries of the groups
RS = [TB[g] * P for g in range(NGRP)]     # row range [RS, RE) of group g
RE = [min(TB[g + 1] * P, NPER) for g in range(NGRP)]
RG = [RE[g] - RS[g] for g in range(NGRP)]
assert max(RG) * NCORES < 32768  # gather idxs are int16

MAXIDX = 1024    # max idxs per dma_gather call (larger calls fault)
NQ = 4           # SWDGE queues to cycle gather calls over

LAST_RESULT = None  # BassKernelResults of the most recent run (for test.py)


def _r16(n):
    return (int(n) + 15) // 16 * 16


def _preprocess(edge_index, x, W1, b1, W2, b2, Wo, bo):
    """Host-side graph preprocessing -> per-core input arrays + schedule."""
    src_e = np.asarray(edge_index[0], np.int64)
    dst_e = np.asarray(edge_index[1], np.int64)
    # degree includes the self loop
    deg = (np.bincount(dst_e, minlength=N) + 1).astype(np.float32)
    disqrt = (1.0 / np.sqrt(deg)).astype(np.float32)

    xh = np.asarray(x, np.float32).astype(np.float16)

    # per (core, tile) edge buckets (no self loops), and per-group subsets
    per_core = []    # [c][t] -> (srcs, dds)  (layer-1 flat order)
    per_core_g = []  # [c][t][g] -> (srcs, pos_in_table, dds)
    n1 = np.zeros((NCORES, NT), np.int64)
    ng = np.zeros((NCORES, NT, NGRP), np.int64)
    for c in range(NCORES):
        m = (dst_e >= c * NPER) & (dst_e < (c + 1) * NPER)
        s_c = src_e[m]
        d_c = dst_e[m] - c * NPER
        order = np.argsort(d_c, kind="stable")
        s_c, d_c = s_c[order], d_c[order]
        bounds = np.searchsorted(d_c, np.arange(0, NT + 1) * P)
        tiles, tiles_g = [], []
        for t in range(NT):
            ss = s_c[bounds[t]:bounds[t + 1]]
            dd = d_c[bounds[t]:bounds[t + 1]] - t * P
            tiles.append((ss, dd))
            n1[c, t] = len(ss)
            cc, rr = ss // NPER, ss % NPER
            gl = []
            for g in range(NGRP):
                sel = (rr >= RS[g]) & (rr < RE[g])
                pos = cc[sel] * RG[g] + (rr[sel] - RS[g])
                gl.append((ss[sel], pos, dd[sel]))
                ng[c, t, g] = sel.sum()
            tiles_g.append(gl)
        per_core.append(tiles)
        per_core_g.append(tiles_g)

    # layer-1 schedule: one group per tile, block-padded
    M1 = [int(n1[:, t].max()) for t in range(NT)]
    B1 = [(m + P - 1) // P for m in M1]
    NB1 = int(sum(B1))
    # layer-2 schedule: NGRP groups per tile, 16-granular idx padding
    MG = [[_r16(ng[:, t, g].max()) for g in range(NGRP)] for t in range(NT)]
    BG = [[(MG[t][g] + P - 1) // P for g in range(NGRP)] for t in range(NT)]
    NB2 = int(sum(sum(bg) for bg in BG))
    NC16 = int(sum(sum(mg) for mg in MG)) // 16
    tile_ws = [min(P, NPER - t * P) for t in range(NT)]

    in_maps = []
    for c in range(NCORES):
        # ---- layer 1: pregathered messages interleaved with S blocks -----
        gs1 = np.zeros((P, NB1, C_IN + P), np.float16)
        blk = 0
        for t in range(NT):
            ss, dd = per_core[c][t]
            nb = B1[t]
            if nb == 0:
                continue
            n = len(ss)
            w = disqrt[dd + t * P + c * NPER] * disqrt[ss]
            ei = np.arange(n)
            bi = ei // P + blk
            pi = ei % P
            gs1[pi, bi, C_IN + dd] = w.astype(np.float16)
            flat_g = np.zeros((nb * P, C_IN), np.float16)
            flat_g[:n] = xh[ss]
            gs1[:, blk:blk + nb, :C_IN] = \
                flat_g.reshape(nb, P, C_IN).transpose(1, 0, 2)
            blk += nb
        assert blk == NB1

        # ---- layer 2: gather idxs + streamed S blocks --------------------
        idx16 = np.zeros((16, NC16), np.int16)
        sblk = np.zeros((P, NB2, P), np.float16)
        col16 = 0
        blk = 0
        for t in range(NT):
            for g in range(NGRP):
                ss, pos, dd = per_core_g[c][t][g]
                m_pad = MG[t][g]
                nb = BG[t][g]
                if m_pad == 0:
                    continue
                n = len(ss)
                flat_i = np.zeros(m_pad, np.int16)
                flat_i[:n] = pos.astype(np.int16)
                idx16[:, col16:col16 + m_pad // 16] = \
                    flat_i.reshape(m_pad // 16, 16).T
                col16 += m_pad // 16
                w = disqrt[dd + t * P + c * NPER] * disqrt[ss]
                ei = np.arange(n)
                bi = ei // P + blk
                sblk[ei % P, bi, dd] = w.astype(np.float16)
                blk += nb
        assert col16 == NC16 and blk == NB2
        idx_full = np.tile(idx16, (8, 1)).astype(np.int16)

        # self-loop scale: deg^-1 of each dst (deg_isqrt^2)
        dsqnm = np.zeros((P, NT), np.float32)
        for t in range(NT):
            tw = tile_ws[t]
            dv = disqrt[c * NPER + t * P: c * NPER + t * P + tw]
            dsqnm[:tw, t] = dv * dv

        in_maps.append({
            "gs1": gs1,
            "sblk": sblk,
            "xss": xh[c * NPER:(c + 1) * NPER].copy(),
            "idx": idx_full,
            "dsqnm": dsqnm,
            "w1": np.asarray(W1, np.float32).astype(np.float16),
            "w2": np.asarray(W2, np.float32).astype(np.float16),
            "wo": np.asarray(Wo, np.float32).astype(np.float16),
            "b1": np.asarray(b1, np.float32).reshape(HID, 1).copy(),
            "b2": np.asarray(b2, np.float32).reshape(HID, 1).copy(),
            "bo": np.tile(np.asarray(bo, np.float32)[None, :], (P, 1)),
        })

    sched = dict(M1=M1, B1=B1, NB1=NB1, MG=MG, BG=BG, NB2=NB2,
                 NC16=NC16, tile_ws=tile_ws)
    return in_maps, sched


def _build_program(sched):
    import concourse.bass as bass
    import concourse.bacc as bacc
    import concourse.tile as tile
    import concourse.mybir as mybir
    from concourse.masks import make_identity

    f32 = mybir.dt.float32
    f16 = mybir.dt.float16
    i16 = mybir.dt.int16
    M1, B1, NB1 = sched["M1"], sched["B1"], sched["NB1"]
    MG, BG, NB2 = sched["MG"], sched["BG"], sched["NB2"]
    NC16, tile_ws = sched["NC16"], sched["tile_ws"]
    nb1_max = max(B1)
    nb2_max = max(sum(bg) for bg in BG)
    nbg_max = [max(BG[t][g] for t in range(NT)) for g in range(NGRP)]
    W = C_IN + P

    nc = bacc.Bacc("TRN2", target_bir_lowering=False, debug=False,
                   num_devices=NCORES, num_swdge_queues=NQ,
                   dynamic_dma_scratch_size=49152)

    gs1_d = nc.dram_tensor("gs1", [P, NB1, W], f16, kind="ExternalInput")
    sblk_d = nc.dram_tensor("sblk", [P, NB2, P], f16, kind="ExternalInput")
    xss_d = nc.dram_tensor("xss", [NPER, C_IN], f16, kind="ExternalInput")
    idx_d = nc.dram_tensor("idx", [P, NC16], i16, kind="ExternalInput")
    dsqnm_d = nc.dram_tensor("dsqnm", [P, NT], f32, kind="ExternalInput")
    w1_d = nc.dram_tensor("w1", [C_IN, HID], f16, kind="ExternalInput")
    w2_d = nc.dram_tensor("w2", [HID, HID], f16, kind="ExternalInput")
    wo_d = nc.dram_tensor("wo", [HID, C_OUT], f16, kind="ExternalInput")
    b1_d = nc.dram_tensor("b1", [HID, 1], f32, kind="ExternalInput")
    b2_d = nc.dram_tensor("b2", [HID, 1], f32, kind="ExternalInput")
    bo_d = nc.dram_tensor("bo", [P, C_OUT], f32, kind="ExternalInput")
    out_d = nc.dram_tensor("out", [NPER, C_OUT], f32, kind="ExternalOutput")

    with tile.TileContext(nc) as tc:
        with tc.tile_pool(name="const", bufs=1) as cpool, \
             tc.tile_pool(name="gs1p", bufs=4) as gs1pool, \
             tc.tile_pool(name="gl0", bufs=8) as gp0, \
             tc.tile_pool(name="gl1", bufs=8) as gp1, \
             tc.tile_pool(name="swide", bufs=4) as swpool, \
             tc.tile_pool(name="work", bufs=6) as wpool, \
             tc.tile_pool(name="psA", bufs=4, space="PSUM") as psA, \
             tc.tile_pool(name="psH", bufs=2, space="PSUM") as psH, \
             tc.tile_pool(name="psT", bufs=2, space="PSUM") as psT, \
             tc.tile_pool(name="dram", bufs=1, space="DRAM") as dram:
            gpools = [gp0, gp1]

            def cload(name, dram_t, shape, dt):
                t = cpool.tile(shape, dt, name=name)
                nc.sync.dma_start(t[:], dram_t[tuple(slice(0, s) for s in shape)])
                return t

            idx_sb = cload("idx_sb", idx_d, [P, NC16], i16)
            dsqnm_sb = cload("dsqnm_sb", dsqnm_d, [P, NT], f32)
            w1_sb = cload("w1_sb", w1_d, [C_IN, HID], f16)
            w2_sb = cload("w2_sb", w2_d, [HID, HID], f16)
            wo_sb = cload("wo_sb", wo_d, [HID, C_OUT], f16)
            b1_sb = cload("b1_sb", b1_d, [HID, 1], f32)
            b2_sb = cload("b2_sb", b2_d, [HID, 1], f32)
            bo_sb = cload("bo_sb", bo_d, [P, C_OUT], f32)

            ident_sb = cpool.tile([P, P], f16, name="ident_sb")
            make_identity(nc, ident_sb[:])

            # prebuilt diagonal bank: diag(deg^-1) per tile, built once on
            # the ScalarEngine and reused by both layers' self-loop matmuls
            dbank = cpool.tile([P, NT, P], f16, name="dbank")
            for t in range(NT):
                nc.scalar.activation(dbank[:, t, :], ident_sb[:, :],
                                     mybir.ActivationFunctionType.Copy,
                                     scale=dsqnm_sb[:, t:t + 1])

            h1s = dram.tile([NPER, HID], f16, name="h1s")
            h1f = [dram.tile([NCORES * RG[g], HID], f16, name=f"h1f{g}",
                             addr_space="Shared") for g in range(NGRP)]

            # register cache for num_idxs_reg values
            regs = {}

            def reg_of(v):
                if v not in regs:
                    regs[v] = nc.gpsimd.to_reg(v)
                return regs[v]

            qctr = [0]

            def next_q():
                q = qctr[0] % NQ
                qctr[0] += 1
                return q

            def tail_a(t, tw, pa, w_sb, b_sb):
                """agg copy + weight matmul + relu (PE stays busy on the
                next tile's aggregation while DVE/ScalarE run these)."""
                agg = wpool.tile([P, tw], f16, tag="agg", name="agg")
                nc.vector.tensor_copy(agg[:], pa[:])
                ph = psH.tile([P, tw], f32, tag="ph", name="ph")
                nc.tensor.matmul(ph[:], lhsT=w_sb[:], rhs=agg[:],
                                 start=True, stop=True)
                h = wpool.tile([P, tw], f16, tag="h", name="h")
                nc.scalar.activation(h[:], ph[:],
                                     mybir.ActivationFunctionType.Relu,
                                     bias=b_sb[:, 0:1])
                return h

            def tail_b(t, tw, h, phase):
                if phase == 0:
                    pt = psT.tile([P, P], f16, tag="pt", name="pt")
                    nc.tensor.transpose(out=pt[:tw, :], in_=h[:, :tw],
                                        identity=ident_sb[:])
                    hn = wpool.tile([P, P], f16, tag="hn", name="hn")
                    nc.vector.tensor_copy(hn[:tw, :], pt[:tw, :])
                    nc.sync.dma_start(h1s[t * P:t * P + tw, :], hn[:tw, :])
                    # fire the AllGather chunk as soon as its rows are done
                    for g in range(NGRP):
                        if t == TB[g + 1] - 1:
                            nc.gpsimd.collective_compute(
                                "AllGather", mybir.AluOpType.bypass,
                                replica_groups=[list(range(NCORES))],
                                ins=[h1s[RS[g]:RE[g], :].opt()],
                                outs=[h1f[g][:].opt()])
                else:
                    po = psT.tile([P, C_OUT], f32, tag="pt", name="po")
                    nc.tensor.matmul(po[:tw, :], lhsT=h[:, :tw],
                                     rhs=wo_sb[:], start=True, stop=True)
                    ob = wpool.tile([P, C_OUT], f32, tag="ob", name="ob")
                    nc.vector.tensor_tensor(out=ob[:tw, :], in0=po[:tw, :],
                                            in1=bo_sb[:tw, :],
                                            op=mybir.AluOpType.add)
                    nc.sync.dma_start(out_d[t * P:t * P + tw, :],
                                      ob[:tw, :])

            def agg0(t, blk):
                tw = tile_ws[t]
                nblk = B1[t]
                GS = gs1pool.tile([P, nb1_max, W], f16, tag="GS", name="GS")
                nc.sync.dma_start(GS[:, 0:nblk, :],
                                  gs1_d[:, blk:blk + nblk, :])
                pa = psA.tile([P, tw], f32, tag="pa", name="pa")
                slab = wpool.tile([P, C_IN], f16, tag="slab", name="slab")
                nc.scalar.dma_start(slab[:tw, :], xss_d[t * P:t * P + tw, :])
                nc.tensor.matmul(pa[:], lhsT=slab[:tw, :],
                                 rhs=dbank[:tw, t, :tw], start=True,
                                 stop=False)
                ks = [P] * (nblk - 1) + [M1[t] - (nblk - 1) * P]
                for j in range(nblk):
                    nc.tensor.matmul(pa[:], lhsT=GS[:ks[j], j, 0:C_IN],
                                     rhs=GS[:ks[j], j, C_IN:C_IN + tw],
                                     start=False, stop=(j == nblk - 1))
                return pa, blk + nblk

            def agg1(t, blk, col16):
                tw = tile_ws[t]
                Gs = []
                for g in range(NGRP):
                    m_pad = MG[t][g]
                    Gg = gpools[g].tile([P, nbg_max[g], C_IN], f16,
                                        tag=f"G{g}", name=f"G{g}")
                    Gs.append(Gg)
                    for o in range(0, m_pad, MAXIDX):
                        n_call = min(MAXIDX, m_pad - o)
                        c0 = col16 + o // 16
                        nc.gpsimd.dma_gather(
                            out_ap=Gg[:, o // P:o // P + (n_call + P - 1) // P, :],
                            in_ap=h1f[g][:, :],
                            idxs_ap=idx_sb[:, c0:c0 + (n_call + 15) // 16],
                            num_idxs=n_call,
                            num_idxs_reg=reg_of(n_call),
                            elem_size=C_IN,
                            queue_num=next_q())
                    col16 += m_pad // 16
                nblk = sum(BG[t])
                swide = swpool.tile([P, nb2_max, P], f16, tag="SW", name="SW")
                nc.scalar.dma_start(swide[:, 0:nblk, :],
                                    sblk_d[:, blk:blk + nblk, :])
                pa = psA.tile([P, tw], f32, tag="pa", name="pa")
                slab = wpool.tile([P, C_IN], f16, tag="slab", name="slab")
                nc.scalar.dma_start(slab[:tw, :], h1s[t * P:t * P + tw, :])
                nc.tensor.matmul(pa[:], lhsT=slab[:tw, :],
                                 rhs=dbank[:tw, t, :tw], start=True,
                                 stop=False)
                j = 0
                for g in range(NGRP):
                    nb = BG[t][g]
                    for jj in range(nb):
                        k = P if jj < nb - 1 else MG[t][g] - (nb - 1) * P
                        nc.tensor.matmul(pa[:], lhsT=Gs[g][:k, jj, :],
                                         rhs=swide[:k, j, :tw],
                                         start=False, stop=(j == nblk - 1))
                        j += 1
                return pa, blk + nblk

            def layer(phase):
                """Tiles are processed in groups of 3: three aggregation
                chains run back-to-back on the PE (a >3.4us busy stretch,
                enough to un-throttle the PE HAM), then the three tails
                follow with their DVE/ScalarE latencies overlapping each
                other's PE work."""
                w_sb = w1_sb if phase == 0 else w2_sb
                b_sb = b1_sb if phase == 0 else b2_sb
                blk = 0
                col16 = 0
                for t0_ in range(0, NT, 3):
                    group = range(t0_, min(t0_ + 3, NT))
                    pas = []
                    for u in group:
                        if phase == 0:
                            pa, blk = agg0(u, blk)
                        else:
                            pa, blk = agg1(u, blk, col16)
                            col16 += sum(MG[u]) // 16
                        pas.append(pa)
                    hs = [tail_a(u, tile_ws[u], pa, w_sb, b_sb)
                          for u, pa in zip(group, pas)]
                    for u, h in zip(group, hs):
                        tail_b(u, tile_ws[u], h, phase)

            layer(0)
            layer(1)

    nc.compile()
    return nc


def kernel(x, edge_index, W1, b1, W2, b2, Wo, bo):
    global LAST_RESULT
    from concourse import bass_utils

    in_maps, sched = _preprocess(edge_index, x, W1, b1, W2, b2, Wo, bo)
    nc = _build_program(sched)
    res = bass_utils.run_bass_kernel_spmd(nc, in_maps,
                                          core_ids=list(range(NCORES)))
    LAST_RESULT = res
    out = np.concatenate([res.results[c]["out"] for c in range(NCORES)], axis=0)
    return out.astype(np.float32)


# revision 12
# speedup vs baseline: 1.0556x; 1.0070x over previous
"""Multi-layer GCN (2x GCNConv + linear head) on 8 Trainium2 NeuronCores.

Strategy (graph/data parallel, node-sharded):
  - Nodes are partitioned contiguously across the 8 cores (6250 each).
  - Each core aggregates messages for its own dst nodes. Edges are bucketed
    by dst tile (128 dsts) on the host and sorted within the tile.
  - The scatter-add (segment sum) runs on the TensorEngine: for each
    128-edge block, a host-built one-hot scatter matrix S[e, d] =
    (d == dst_e) * w_e (w_e = deg_isqrt[src] * deg_isqrt[dst], the full GCN
    normalization) is contracted against the message block; PSUM
    accumulates across blocks into a feature-major agg^T tile. All S
    matrices are precomputed on the host and streamed from HBM, so the
    TensorEngine consumes whole tiles with one semaphore per tile and no
    per-block compute-engine dependency (keeps the PE HAM un-throttled).
  - Layer 1 needs no on-device gather: source messages are pre-gathered on
    the host and interleaved with their S blocks in one [128, NB1, 256] f16
    tensor (G row ++ S row), streamed per tile with a single HWDGE DMA.
  - Layer 2 gathers h1 rows with dma_gather (batched indirect DMA) from
    AllGathered tables in HBM, with gather calls cycled over all 4 SWDGE
    queues so Q7 descriptor generation uses all 8 GpSimd cores.
  - Self loops: each tile's own rows are a contiguous HWDGE DMA load,
    accumulated via a matmul against a prebuilt diagonal bank
    diag(deg^-1[dst]) (49 tiles built once on the ScalarEngine).
  - The inter-layer AllGather is split in 2 by source-node range, each
    chunk fired as soon as layer 1 finishes its tiles, so collectives and
    layer-2 gather descriptor generation overlap layer 1's tail. Layer-2
    edges are grouped by source range into 2 groups per tile; each group's
    dma_gather indexes its own table (int16-safe).
  - Layer weights are applied right on the feature-major agg tiles; layer-1
    output is transposed back to node-major (TensorE transpose) and written
    to HBM for the chunked AllGather. Layer-2 output stays feature-major
    and feeds the output projection directly (lhsT = h2^T), producing
    node-major [dst, 64] tiles.
"""

import sys

sys.path.insert(0, "/opt/trn_rl_repo")

import numpy as np

N = 50000
C_IN = 128
HID = 128
C_OUT = 64
NCORES = 8
NPER = N // NCORES
P = 128
NT = (NPER + P - 1) // P

NGRP = 2
TB = [0, 24, NT]                          # tile boundaries of the groups
RS = [TB[g] * P for g in range(NGRP)]     # row range [RS, RE) of group g
RE = [min(TB[g + 1] * P, NPER) for g in range(NGRP)]
RG = [RE[g] - RS[g] for g in range(NGRP)]
assert max(RG) * NCORES < 32768  # gather idxs are int16

MAXIDX = 1024    # max idxs per dma_gather call (larger calls fault)
NQ = 4           # SWDGE queues to cycle gather calls over

LAST_RESULT = None  # BassKernelResults of the most recent run (for test.py)


def _r16(n):
    return (int(n) + 15) // 16 * 16


def _preprocess(edge_index, x, W1, b1, W2, b2, Wo, bo):
    """Host-side graph preprocessing -> per-core input arrays + schedule."""
    src_e = np.asarray(edge_index[0], np.int64)
    dst_e = np.asarray(edge_index[1], np.int64)
    # degree includes the self loop
    deg = (np.bincount(dst_e, minlength=N) + 1).astype(np.float32)
    disqrt = (1.0 / np.sqrt(deg)).astype(np.float32)

    xh = np.asarray(x, np.float32).astype(np.float16)

    # per (core, tile) edge buckets (no self loops), and per-group subsets
    per_core = []    # [c][t] -> (srcs, dds)  (layer-1 flat order)
    per_core_g = []  # [c][t][g] -> (srcs, pos_in_table, dds)
    n1 = np.zeros((NCORES, NT), np.int64)
    ng = np.zeros((NCORES, NT, NGRP), np.int64)
    for c in range(NCORES):
        m = (dst_e >= c * NPER) & (dst_e < (c + 1) * NPER)
        s_c = src_e[m]
        d_c = dst_e[m] - c * NPER
        order = np.argsort(d_c, kind="stable")
        s_c, d_c = s_c[order], d_c[order]
        bounds = np.searchsorted(d_c, np.arange(0, NT + 1) * P)
        tiles, tiles_g = [], []
        for t in range(NT):
            ss = s_c[bounds[t]:bounds[t + 1]]
            dd = d_c[bounds[t]:bounds[t + 1]] - t * P
            tiles.append((ss, dd))
            n1[c, t] = len(ss)
            cc, rr = ss // NPER, ss % NPER
            gl = []
            for g in range(NGRP):
                sel = (rr >= RS[g]) & (rr < RE[g])
                pos = cc[sel] * RG[g] + (rr[sel] - RS[g])
                gl.append((ss[sel], pos, dd[sel]))
                ng[c, t, g] = sel.sum()
            tiles_g.append(gl)
        per_core.append(tiles)
        per_core_g.append(tiles_g)

    # layer-1 schedule: one group per tile, block-padded
    M1 = [int(n1[:, t].max()) for t in range(NT)]
    B1 = [(m + P - 1) // P for m in M1]
    NB1 = int(sum(B1))
    # layer-2 schedule: NGRP groups per tile, 16-granular idx padding
    MG = [[_r16(ng[:, t, g].max()) for g in range(NGRP)] for t in range(NT)]
    BG = [[(MG[t][g] + P - 1) // P for g in range(NGRP)] for t in range(NT)]
    NB2 = int(sum(sum(bg) for bg in BG))
    NC16 = int(sum(sum(mg) for mg in MG)) // 16
    tile_ws = [min(P, NPER - t * P) for t in range(NT)]

    in_maps = []
    for c in range(NCORES):
        # ---- layer 1: pregathered messages interleaved with S blocks -----
        gs1 = np.zeros((P, NB1, C_IN + P), np.float16)
        blk = 0
        for t in range(NT):
            ss, dd = per_core[c][t]
            nb = B1[t]
            if nb == 0:
                continue
            n = len(ss)
            w = disqrt[dd + t * P + c * NPER] * disqrt[ss]
            ei = np.arange(n)
            bi = ei // P + blk
            pi = ei % P
            gs1[pi, bi, C_IN + dd] = w.astype(np.float16)
            flat_g = np.zeros((nb * P, C_IN), np.float16)
            flat_g[:n] = xh[ss]
            gs1[:, blk:blk + nb, :C_IN] = \
                flat_g.reshape(nb, P, C_IN).transpose(1, 0, 2)
            blk += nb
        assert blk == NB1

        # ---- layer 2: gather idxs + streamed S blocks --------------------
        idx16 = np.zeros((16, NC16), np.int16)
        sblk = np.zeros((P, NB2, P), np.float16)
        col16 = 0
        blk = 0
        for t in range(NT):
            for g in range(NGRP):
                ss, pos, dd = per_core_g[c][t][g]
                m_pad = MG[t][g]
                nb = BG[t][g]
                if m_pad == 0:
                    continue
                n = len(ss)
                flat_i = np.zeros(m_pad, np.int16)
                flat_i[:n] = pos.astype(np.int16)
                idx16[:, col16:col16 + m_pad // 16] = \
                    flat_i.reshape(m_pad // 16, 16).T
                col16 += m_pad // 16
                w = disqrt[dd + t * P + c * NPER] * disqrt[ss]
                ei = np.arange(n)
                bi = ei // P + blk
                sblk[ei % P, bi, dd] = w.astype(np.float16)
                blk += nb
        assert col16 == NC16 and blk == NB2
        idx_full = np.tile(idx16, (8, 1)).astype(np.int16)

        # self-loop scale: deg^-1 of each dst (deg_isqrt^2)
        dsqnm = np.zeros((P, NT), np.float32)
        for t in range(NT):
            tw = tile_ws[t]
            dv = disqrt[c * NPER + t * P: c * NPER + t * P + tw]
            dsqnm[:tw, t] = dv * dv

        in_maps.append({
            "gs1": gs1,
            "sblk": sblk,
            "xss": xh[c * NPER:(c + 1) * NPER].copy(),
            "idx": idx_full,
            "dsqnm": dsqnm,
            "w1": np.asarray(W1, np.float32).astype(np.float16),
            "w2": np.asarray(W2, np.float32).astype(np.float16),
            "wo": np.asarray(Wo, np.float32).astype(np.float16),
            "b1": np.asarray(b1, np.float32).reshape(HID, 1).copy(),
            "b2": np.asarray(b2, np.float32).reshape(HID, 1).copy(),
            "bo": np.tile(np.asarray(bo, np.float32)[None, :], (P, 1)),
        })

    sched = dict(M1=M1, B1=B1, NB1=NB1, MG=MG, BG=BG, NB2=NB2,
                 NC16=NC16, tile_ws=tile_ws)
    return in_maps, sched


def _build_program(sched):
    import concourse.bass as bass
    import concourse.bacc as bacc
    import concourse.tile as tile
    import concourse.mybir as mybir
    from concourse.masks import make_identity

    f32 = mybir.dt.float32
    f16 = mybir.dt.float16
    i16 = mybir.dt.int16
    M1, B1, NB1 = sched["M1"], sched["B1"], sched["NB1"]
    MG, BG, NB2 = sched["MG"], sched["BG"], sched["NB2"]
    NC16, tile_ws = sched["NC16"], sched["tile_ws"]
    nb1_max = max(B1)
    nb2_max = max(sum(bg) for bg in BG)
    nbg_max = [max(BG[t][g] for t in range(NT)) for g in range(NGRP)]
    W = C_IN + P

    nc = bacc.Bacc("TRN2", target_bir_lowering=False, debug=False,
                   num_devices=NCORES, num_swdge_queues=NQ,
                   dynamic_dma_scratch_size=49152)

    gs1_d = nc.dram_tensor("gs1", [P, NB1, W], f16, kind="ExternalInput")
    sblk_d = nc.dram_tensor("sblk", [P, NB2, P], f16, kind="ExternalInput")
    xss_d = nc.dram_tensor("xss", [NPER, C_IN], f16, kind="ExternalInput")
    idx_d = nc.dram_tensor("idx", [P, NC16], i16, kind="ExternalInput")
    dsqnm_d = nc.dram_tensor("dsqnm", [P, NT], f32, kind="ExternalInput")
    w1_d = nc.dram_tensor("w1", [C_IN, HID], f16, kind="ExternalInput")
    w2_d = nc.dram_tensor("w2", [HID, HID], f16, kind="ExternalInput")
    wo_d = nc.dram_tensor("wo", [HID, C_OUT], f16, kind="ExternalInput")
    b1_d = nc.dram_tensor("b1", [HID, 1], f32, kind="ExternalInput")
    b2_d = nc.dram_tensor("b2", [HID, 1], f32, kind="ExternalInput")
    bo_d = nc.dram_tensor("bo", [P, C_OUT], f32, kind="ExternalInput")
    out_d = nc.dram_tensor("out", [NPER, C_OUT], f32, kind="ExternalOutput")

    with tile.TileContext(nc) as tc:
        with tc.tile_pool(name="const", bufs=1) as cpool, \
             tc.tile_pool(name="gs1p", bufs=4) as gs1pool, \
             tc.tile_pool(name="gl0", bufs=8) as gp0, \
             tc.tile_pool(name="gl1", bufs=8) as gp1, \
             tc.tile_pool(name="swide", bufs=4) as swpool, \
             tc.tile_pool(name="work", bufs=6) as wpool, \
             tc.tile_pool(name="psA", bufs=4, space="PSUM") as psA, \
             tc.tile_pool(name="psH", bufs=2, space="PSUM") as psH, \
             tc.tile_pool(name="psT", bufs=2, space="PSUM") as psT, \
             tc.tile_pool(name="dram", bufs=1, space="DRAM") as dram:
            gpools = [gp0, gp1]

            def cload(name, dram_t, shape, dt):
                t = cpool.tile(shape, dt, name=name)
                nc.sync.dma_start(t[:], dram_t[tuple(slice(0, s) for s in shape)])
                return t

            idx_sb = cload("idx_sb", idx_d, [P, NC16], i16)
            dsqnm_sb = cload("dsqnm_sb", dsqnm_d, [P, NT], f32)
            w1_sb = cload("w1_sb", w1_d, [C_IN, HID], f16)
            w2_sb = cload("w2_sb", w2_d, [HID, HID], f16)
            wo_sb = cload("wo_sb", wo_d, [HID, C_OUT], f16)
            b1_sb = cload("b1_sb", b1_d, [HID, 1], f32)
            b2_sb = cload("b2_sb", b2_d, [HID, 1], f32)
            bo_sb = cload("bo_sb", bo_d, [P, C_OUT], f32)

            ident_sb = cpool.tile([P, P], f16, name="ident_sb")
            make_identity(nc, ident_sb[:])

            # prebuilt diagonal bank: diag(deg^-1) per tile, built once on
            # the ScalarEngine and reused by both layers' self-loop matmuls
            dbank = cpool.tile([P, NT, P], f16, name="dbank")
            for t in range(NT):
                nc.scalar.activation(dbank[:, t, :], ident_sb[:, :],
                                     mybir.ActivationFunctionType.Copy,
                                     scale=dsqnm_sb[:, t:t + 1])

            h1s = dram.tile([NPER, HID], f16, name="h1s")
            h1f = [dram.tile([NCORES * RG[g], HID], f16, name=f"h1f{g}",
                             addr_space="Shared") for g in range(NGRP)]

            # register cache for num_idxs_reg values
            regs = {}

            def reg_of(v):
                if v not in regs:
                    regs[v] = nc.gpsimd.to_reg(v)
                return regs[v]

            qctr = [0]

            def next_q():
                q = qctr[0] % NQ
                qctr[0] += 1
                return q

            def tail_a(t, tw, pa, w_sb, b_sb):
                """agg copy + weight matmul + relu (PE stays busy on the
                next tile's aggregation while DVE/ScalarE run these)."""
                agg = wpool.tile([P, tw], f16, tag="agg", name="agg")
                nc.vector.tensor_copy(agg[:], pa[:])
                ph = psH.tile([P, tw], f32, tag="ph", name="ph")
                nc.tensor.matmul(ph[:], lhsT=w_sb[:], rhs=agg[:],
                                 start=True, stop=True)
                h = wpool.tile([P, tw], f16, tag="h", name="h")
                nc.scalar.activation(h[:], ph[:],
                                     mybir.ActivationFunctionType.Relu,
                                     bias=b_sb[:, 0:1])
                return h

            def tail_b(t, tw, h, phase):
                if phase == 0:
                    pt = psT.tile([P, P], f16, tag="pt", name="pt")
                    nc.tensor.transpose(out=pt[:tw, :], in_=h[:, :tw],
                                        identity=ident_sb[:])
                    hn = wpool.tile([P, P], f16, tag="hn", name="hn")
                    nc.vector.tensor_copy(hn[:tw, :], pt[:tw, :])
                    nc.sync.dma_start(h1s[t * P:t * P + tw, :], hn[:tw, :])
                    # fire the AllGather chunk as soon as its rows are done
                    for g in range(NGRP):
                        if t == TB[g + 1] - 1:
                            nc.gpsimd.collective_compute(
                                "AllGather", mybir.AluOpType.bypass,
                                replica_groups=[list(range(NCORES))],
                                ins=[h1s[RS[g]:RE[g], :].opt()],
                                outs=[h1f[g][:].opt()])
                else:
                    po = psT.tile([P, C_OUT], f32, tag="pt", name="po")
                    nc.tensor.matmul(po[:tw, :], lhsT=h[:, :tw],
                                     rhs=wo_sb[:], start=True, stop=True)
                    ob = wpool.tile([P, C_OUT], f32, tag="ob", name="ob")
                    nc.vector.tensor_tensor(out=ob[:tw, :], in0=po[:tw, :],
                                            in1=bo_sb[:tw, :],
                                            op=mybir.AluOpType.add)
                    nc.sync.dma_start(out_d[t * P:t * P + tw, :],
                                      ob[:tw, :])

            def agg0(t, blk):
                tw = tile_ws[t]
                nblk = B1[t]
                # alternate the two HWDGE rings: they issue FIFO-serial per
                # ring, and the big gs1 streams would otherwise serialize
                eng, oth = (nc.sync, nc.scalar) if t % 2 == 0 else \
                           (nc.scalar, nc.sync)
                GS = gs1pool.tile([P, nb1_max, W], f16, tag="GS", name="GS")
                eng.dma_start(GS[:, 0:nblk, :],
                              gs1_d[:, blk:blk + nblk, :])
                pa = psA.tile([P, tw], f32, tag="pa", name="pa")
                slab = wpool.tile([P, C_IN], f16, tag="slab", name="slab")
                oth.dma_start(slab[:tw, :], xss_d[t * P:t * P + tw, :])
                nc.tensor.matmul(pa[:], lhsT=slab[:tw, :],
                                 rhs=dbank[:tw, t, :tw], start=True,
                                 stop=False)
                ks = [P] * (nblk - 1) + [M1[t] - (nblk - 1) * P]
                for j in range(nblk):
                    nc.tensor.matmul(pa[:], lhsT=GS[:ks[j], j, 0:C_IN],
                                     rhs=GS[:ks[j], j, C_IN:C_IN + tw],
                                     start=False, stop=(j == nblk - 1))
                return pa, blk + nblk

            def agg1(t, blk, col16):
                tw = tile_ws[t]
                Gs = []
                for g in range(NGRP):
                    m_pad = MG[t][g]
                    Gg = gpools[g].tile([P, nbg_max[g], C_IN], f16,
                                        tag=f"G{g}", name=f"G{g}")
                    Gs.append(Gg)
                    for o in range(0, m_pad, MAXIDX):
                        n_call = min(MAXIDX, m_pad - o)
                        c0 = col16 + o // 16
                        nc.gpsimd.dma_gather(
                            out_ap=Gg[:, o // P:o // P + (n_call + P - 1) // P, :],
                            in_ap=h1f[g][:, :],
                            idxs_ap=idx_sb[:, c0:c0 + (n_call + 15) // 16],
                            num_idxs=n_call,
                            num_idxs_reg=reg_of(n_call),
                            elem_size=C_IN,
                            queue_num=next_q())
                    col16 += m_pad // 16
                nblk = sum(BG[t])
                eng, oth = (nc.sync, nc.scalar) if t % 2 == 0 else \
                           (nc.scalar, nc.sync)
                swide = swpool.tile([P, nb2_max, P], f16, tag="SW", name="SW")
                eng.dma_start(swide[:, 0:nblk, :],
                              sblk_d[:, blk:blk + nblk, :])
                pa = psA.tile([P, tw], f32, tag="pa", name="pa")
                slab = wpool.tile([P, C_IN], f16, tag="slab", name="slab")
                oth.dma_start(slab[:tw, :], h1s[t * P:t * P + tw, :])
                nc.tensor.matmul(pa[:], lhsT=slab[:tw, :],
                                 rhs=dbank[:tw, t, :tw], start=True,
                                 stop=False)
                j = 0
                for g in range(NGRP):
                    nb = BG[t][g]
                    for jj in range(nb):
                        k = P if jj < nb - 1 else MG[t][g] - (nb - 1) * P
                        nc.tensor.matmul(pa[:], lhsT=Gs[g][:k, jj, :],
                                         rhs=swide[:k, j, :tw],
                                         start=False, stop=(j == nblk - 1))
                        j += 1
                return pa, blk + nblk

            def layer(phase):
                """Tiles are processed in groups of 3: three aggregation
                chains run back-to-back on the PE (a >3.4us busy stretch,
                enough to un-throttle the PE HAM), then the three tails
                follow with their DVE/ScalarE latencies overlapping each
                other's PE work."""
                w_sb = w1_sb if phase == 0 else w2_sb
                b_sb = b1_sb if phase == 0 else b2_sb
                blk = 0
                col16 = 0
                for t0_ in range(0, NT, 3):
                    group = range(t0_, min(t0_ + 3, NT))
                    pas = []
                    for u in group:
                        if phase == 0:
                            pa, blk = agg0(u, blk)
                        else:
                            pa, blk = agg1(u, blk, col16)
                            col16 += sum(MG[u]) // 16
                        pas.append(pa)
                    hs = [tail_a(u, tile_ws[u], pa, w_sb, b_sb)
                          for u, pa in zip(group, pas)]
                    for u, h in zip(group, hs):
                        tail_b(u, tile_ws[u], h, phase)

            layer(0)
            layer(1)

    nc.compile()
    return nc


def kernel(x, edge_index, W1, b1, W2, b2, Wo, bo):
    global LAST_RESULT
    from concourse import bass_utils

    in_maps, sched = _preprocess(edge_index, x, W1, b1, W2, b2, Wo, bo)
    nc = _build_program(sched)
    res = bass_utils.run_bass_kernel_spmd(nc, in_maps,
                                          core_ids=list(range(NCORES)))
    LAST_RESULT = res
    out = np.concatenate([res.results[c]["out"] for c in range(NCORES)], axis=0)
    return out.astype(np.float32)
